# revision 57
# baseline (speedup 1.0000x reference)
"""Trainium2 Bass kernel for nn_Attention_55130200211640 (v3).

GQA attention block: q/k/v projections + RoPE (theta=1e6) + non-causal
softmax attention (16 q-heads, 4 kv-heads, head_dim 128) + output
projection. B=4, S=2048, HID=2048, fp32 I/O.

Sharding: (batch x 4) x (kv-group-half x 2) = 8 cores, tensor-parallel
over heads within a batch. Each core computes, for its batch, the full
2048-token sequence restricted to 2 of the 4 kv groups (= 8 of the 16
q heads): q/k/v projections, RoPE, attention, and a PARTIAL o_proj
(contraction over its 8 heads only). The host adds the two partial y's
per batch.

v3: all four projections (q/k/v/o) run as fp8e4 DoubleRow matmuls with
hi+lo operand splitting. Each operand t is stored as t_hi = fp8(s*t),
t_lo = fp8(s*t - t_hi); one DoubleRow pass computes two 128-contraction
products at 0.5 cycles/out-col, and per k-tile-pair the three passes
  (w1_hi,w2_hi)x(x1_hi,x2_hi) + (w1_lo,w2_lo)x(x1_hi,x2_hi)
                              + (w1_hi,w2_hi)x(x1_lo,x2_lo)
reconstruct w.x to ~bf16 accuracy (lo.lo dropped) at 0.75x the bf16
column cost. The q-projection additionally drops the x_lo pass on half
its k-tile pairs (skip_hl): +1.1e-2 deterministic L2 error (measured
total 1.70e-2 vs the 2e-2 gate) for 1/6 of the q-proj PE time.
Scores and AV stay bf16 (E/K/Q splits don't pay there).
Scale bookkeeping: x scaled by AX, wq/wk/wv by BW -> exp scale folds
1/(AX^2 BW^2); V path carries AX*BW; OT split scaled by EO (folded as
1/EO into the ones vector so rz = EO/Z); wo scaled by GW; final y
eviction multiplies by 1/(AX*BW*EO*GW).

Per-core dataflow ("contraction-on-partition" layouts everywhere):
  preamble: X^T hi/lo fp8 resident (chunk-major for contiguous DMA);
            K^T [d,2|S] and V [j,256] computed (DoubleRow) + roped,
            Q^T head 0.
  slots:    16 slots in U-MAJOR order: all 8 heads' first query half
            (u=0), then all second halves -- so every head's u0 OT is
            done by slot 7 and o_proj prefill never waits on the last
            slot's normalize chain. Per slot: S^T[j,i] = K^T_g . Q^T_h
            on PE (bf16) -> exp on ACT (scale folded) -> E bf16;
            U^T[d,i] = sum_j V E (psum-accumulated, bf16); Z[i] =
            sum_j E via stationary-E matmuls with a [128,1]
            (1/EO)-valued moving operand; Z -> free-axis reduce ->
            recip -> PE transpose -> DRAM-roundtrip flatten -> gpsimd
            broadcast -> tmp = U*(EO/Z) on gpsimd -> OT_hi fp8 (gpsimd
            cast) + OT_lo fp8 (gpsimd subtract; keeping it on Pool
            avoids head-of-line blocking DVE's zred/usb chain). The
            next head's q-projection DoubleRow passes are interleaved
            as PE filler (8 pieces/chunk, one per jt) plus two at the
            slot boundary; the next slot's first two scores are emitted
            at jt 14/15 so its first AV finds its exp done. Slots 14/15
            (no filler left) interleave o_proj groups instead.
  o_proj:   y[i,o] = sum_h OT_h . wo_h via DoubleRow over head pairs,
            psum-accumulated, evicted as bf16 plain copies (the unscale
            factor is folded into the host-side partial-sum gather);
            the final group evicts per 256-half to shorten the drain.
"""

import numpy as np

B, S, HID = 4, 2048, 2048
H, KV, D = 16, 4, 128
N_CORES = 8
HC = 8                 # heads per core
KVC = 2                # kv groups per core
CT = HID // 128        # contraction tiles
CP = CT // 2           # contraction tile pairs (DoubleRow)
JT = S // 128          # key tiles
SCALE = 1.0 / float(np.sqrt(D))

AX = 16.0              # x hi/lo split scale
BW = 800.0             # wq/wk/wv split scale
EO = 1.0 / 320.0       # OT split scale (1/EO exact in bf16)
GW = 800.0             # wo split scale
EXP_SCALE = SCALE / (AX * AX * BW * BW)
Y_SCALE = 1.0 / (AX * BW * EO * GW)

_cache = {}


def _emit(nc, tc, io):
    import concourse.mybir as mybir
    from collections import deque
    from contextlib import ExitStack

    F32 = mybir.dt.float32
    BF16 = mybir.dt.bfloat16
    FP8 = mybir.dt.float8e4
    DR = mybir.MatmulPerfMode.DoubleRow
    Exp = mybir.ActivationFunctionType.Exp
    Copy = mybir.ActivationFunctionType.Copy
    AxX = mybir.AxisListType.X
    Add = mybir.AluOpType.add

    (xh_d, xl_d, cosk_d, sinkm_d, wqh_d, wql_d, wkh_d, wkl_d, wvh_d, wvl_d,
     woh_d, wol_d, ones_d, ident_d, y_d) = io

    ctx = ExitStack()

    # ---------------- persistent SBUF tiles (left heap) ----------------
    const_pool = ctx.enter_context(tc.tile_pool(name="const", bufs=1, side="left"))
    ones_t = const_pool.tile([128, 1], BF16)
    ident_t = const_pool.tile([128, 128], F32)
    COS = const_pool.tile([128, S], BF16)
    SINM = const_pool.tile([128, S], BF16)

    dram_pool = ctx.enter_context(tc.tile_pool(name="drp", bufs=1, space="DRAM"))
    kv_pool = ctx.enter_context(tc.tile_pool(name="kv", bufs=1, side="left"))
    KT = kv_pool.tile([128, KVC, S], BF16)        # [d, g, j]
    VV = kv_pool.tile([128, JT, KVC * 128], BF16)  # [j, jt, g*128+d]
    q_pool = ctx.enter_context(tc.tile_pool(name="qt", bufs=1, side="left"))
    QT = q_pool.tile([128, HC, S], BF16)          # [d, h, i]
    o_pool = ctx.enter_context(tc.tile_pool(name="ot", bufs=1, side="left"))
    OTH = o_pool.tile([128, HC, S], FP8)          # [d, h, i] hi
    OTL = o_pool.tile([128, HC, S], FP8)          # [d, h, i] lo

    # X^T hi/lo resident for the whole projection span. Opened last on
    # the left heap so it can be closed (LIFO) mid-emission to make room
    # for the wo_lo tile.
    x_ctx = ExitStack()
    x_pool = x_ctx.enter_context(tc.tile_pool(name="xp", bufs=1, side="left"))
    # chunk-major so each 256-token chunk DMA is contiguous on both
    # sides (4KB descriptors; sub-512B descriptors pay 2x on the bus).
    XH = x_pool.tile([128, 8, CT, 256], FP8)      # [hid%128, tch, ct, tok%256]
    XL = x_pool.tile([128, 8, CT, 256], FP8)

    w_pool = ctx.enter_context(tc.tile_pool(name="wp", bufs=2, side="right"))
    st_pool = ctx.enter_context(tc.tile_pool(name="st", bufs=1, side="right"))
    e_pool = ctx.enter_context(tc.tile_pool(name="ep", bufs=3, side="right"))
    rz_pool = ctx.enter_context(tc.tile_pool(name="rz", bufs=1, side="right"))

    # Persistent psum pool for projection chunks: 1 bank, used by the
    # q-projection filler during the slots as well.
    pq_ctx = ExitStack()
    p_q = pq_ctx.enter_context(
        tc.tile_pool(name="p_q", bufs=1, space="PSUM", side="left"))

    def rope(ps, c0, n, dst):
        """RoPE a [128, n] psum tile (layout [d, pos], positions c0:c0+n)
        -> bf16 SBUF dst. rotate_half is a cross-partition half-swap; the
        sign lives in SINM (rows 0:64 pre-negated on the host).
        The psum is evicted to bf16 SBUF in ONE copy (so the projection
        psum bank frees ~1.4us earlier for the next chunk) and the rope
        arithmetic runs all-bf16, which qualifies for the DVE 2x mode
        (327ns vs 658ns per [128,512] op)."""
        qsb = st_pool.tile([128, 512], BF16, tag="qsb", bufs=2, name="qsb")
        nc.vector.tensor_copy(qsb[:, 0:n], ps[:])
        rope_ops(qsb, c0, n, dst)

    def rope_ops(qsb, c0, n, dst):
        tmp = st_pool.tile([128, 512], BF16, tag="tmp", bufs=2, name="tmp")
        stage = st_pool.tile([128, 512], BF16, tag="stage", bufs=2,
                             name="stage")
        # SINM halves are pre-swapped on the host so each mul reads both
        # SBUF inputs at the SAME base partition (hw BIR rule); only the
        # OUTPUT partitions are swapped, as in the proven psum variant.
        nc.vector.tensor_mul(stage[0:64, 0:n], qsb[64:128, 0:n],
                             SINM[64:128, c0:c0 + n])
        nc.vector.tensor_mul(stage[64:128, 0:n], qsb[0:64, 0:n],
                             SINM[0:64, c0:c0 + n])
        nc.vector.tensor_mul(tmp[:, 0:n], qsb[:, 0:n], COS[:, c0:c0 + n])
        nc.vector.tensor_add(dst, stage[:, 0:n], tmp[:, 0:n])

    def dr3(ps_sl, wh, wl, xh, xl, first, last, skip_hl=False):
        """Emit the 3 DoubleRow passes for one k-tile pair: hh + lh + hl.
        wh/wl: stationary [128, 2, M]; xh/xl: moving [128, 2, N<=256].
        skip_hl drops the x_lo correction for this pair -- used on 2 of
        8 q-proj pairs to trade ~1.3% extra (deterministic, measured)
        error for 1/12 of the q-projection's PE time."""
        nc.tensor.matmul(ps_sl, wh, xh, start=first, stop=False, perf_mode=DR)
        if skip_hl:
            nc.tensor.matmul(ps_sl, wl, xh, start=False, stop=last,
                             perf_mode=DR)
            return
        nc.tensor.matmul(ps_sl, wl, xh, start=False, stop=False, perf_mode=DR)
        nc.tensor.matmul(ps_sl, wh, xl, start=False, stop=last, perf_mode=DR)

    # ---------------- preamble: K, V, Q(0) projections ----------------
    # DMA priority: the hi tensors for the first chunk's A-passes land
    # first, then the lo tensors, then the X stream stays ahead of the
    # 256-token chunk walk (~5.1us compute / ~3.2us DMA per chunk).
    wkh_ts, wkl_ts = [], []
    for g in range(KVC):
        wkh_t = w_pool.tile([128, CT, 128], FP8, tag="wh", bufs=4,
                            name="wkh_t")
        nc.sync.dma_start(wkh_t[:], wkh_d[g])
        wkh_ts.append(wkh_t)
    nc.sync.dma_start(XH[:, 0], xh_d[0])
    nc.sync.dma_start(XL[:, 0], xl_d[0])
    for g in range(KVC):
        wkl_t = w_pool.tile([128, CT, 128], FP8, tag="wl", bufs=4, name="wkl_t")
        nc.sync.dma_start(wkl_t[:], wkl_d[g])
        wkl_ts.append(wkl_t)
    wvh_t = w_pool.tile([128, CT, 256], FP8, tag="wvh", bufs=1, name="wvh_t")
    nc.sync.dma_start(wvh_t[:], wvh_d[:])
    nc.scalar.dma_start(COS[:, 0:1024], cosk_d[:, 0:1024])
    nc.scalar.dma_start(SINM[:, 0:1024], sinkm_d[:, 0:1024])
    wvl_t = w_pool.tile([128, CT, 256], FP8, tag="wvl", bufs=1, name="wvl_t")
    nc.sync.dma_start(wvl_t[:], wvl_d[:])
    for tch in range(1, 8):
        t0 = tch * 256
        nc.sync.dma_start(XH[:, tch], xh_d[tch])
        nc.sync.dma_start(XL[:, tch], xl_d[tch])
        if tch == 1:
            nc.scalar.dma_start(COS[:, 1024:2048], cosk_d[:, 1024:2048])
            nc.scalar.dma_start(SINM[:, 1024:2048], sinkm_d[:, 1024:2048])
    wqh_t0 = w_pool.tile([128, CT, 128], FP8, tag="wh", bufs=4, name="wqh_t")
    nc.sync.dma_start(wqh_t0[:], wqh_d[0])
    wql_t0 = w_pool.tile([128, CT, 128], FP8, tag="wl", bufs=4, name="wql_t")
    nc.sync.dma_start(wql_t0[:], wql_d[0])
    nc.sync.dma_start(ones_t[:], ones_d[:])
    nc.sync.dma_start(ident_t[:], ident_d[:])

    with tc.tile_pool(name="p_pre", bufs=1, space="PSUM", side="right") as p_pre:
        for tch in range(8):
            j0 = tch * 256
            kps = []
            for g in range(KVC):
                kps.append(p_pre.tile([128, 512], F32, tag="c", bufs=6,
                                      name="ps_pre"))
            vps = []
            for jt in range(tch * 2, tch * 2 + 2):
                vps.append(p_pre.tile([128, 512], F32, tag="c", bufs=6,
                                      name="ps_pre"))
            # Pass order follows first-chunk DMA arrival: K-hi passes,
            # then XL-dependent K passes, then wkl passes + ropes, then V.
            for g in range(KVC):
                for c in range(CP):
                    nc.tensor.matmul(kps[g][:, 0:256],
                                     wkh_ts[g][:, 2 * c:2 * c + 2, :],
                                     XH[:, tch, 2 * c:2 * c + 2, :],
                                     start=(c == 0), stop=False, perf_mode=DR)
            for g in range(KVC):
                for c in range(CP):
                    nc.tensor.matmul(kps[g][:, 0:256],
                                     wkh_ts[g][:, 2 * c:2 * c + 2, :],
                                     XL[:, tch, 2 * c:2 * c + 2, :],
                                     start=False, stop=False, perf_mode=DR)
            for g in range(KVC):
                for c in range(CP):
                    nc.tensor.matmul(kps[g][:, 0:256],
                                     wkl_ts[g][:, 2 * c:2 * c + 2, :],
                                     XH[:, tch, 2 * c:2 * c + 2, :],
                                     start=False, stop=(c == CP - 1),
                                     perf_mode=DR)
                rope(kps[g][:, 0:256], j0, 256, KT[:, g, j0:j0 + 256])
            for i, jt in enumerate(range(tch * 2, tch * 2 + 2)):
                jb = (jt % 2) * 128
                for c in range(CP):
                    nc.tensor.matmul(vps[i][:, 0:256],
                                     XH[:, tch, 2 * c:2 * c + 2, jb:jb + 128],
                                     wvh_t[:, 2 * c:2 * c + 2, :],
                                     start=(c == 0), stop=False, perf_mode=DR)
                for c in range(CP):
                    nc.tensor.matmul(vps[i][:, 0:256],
                                     XL[:, tch, 2 * c:2 * c + 2, jb:jb + 128],
                                     wvh_t[:, 2 * c:2 * c + 2, :],
                                     start=False, stop=False, perf_mode=DR)
                for c in range(CP):
                    nc.tensor.matmul(vps[i][:, 0:256],
                                     XH[:, tch, 2 * c:2 * c + 2, jb:jb + 128],
                                     wvl_t[:, 2 * c:2 * c + 2, :],
                                     start=False, stop=(c == CP - 1),
                                     perf_mode=DR)
                nc.scalar.copy(VV[:, jt, :], vps[i][:, 0:256])
        # Q projection head 0.
        for qc in range(4):
            i0 = qc * 512
            ps = p_pre.tile([128, 512], F32, tag="c", bufs=6, name="ps_pre")
            for half in range(2):
                off = half * 256
                tch = qc * 2 + half
                for c in range(CP):
                    dr3(ps[:, off:off + 256],
                        wqh_t0[:, 2 * c:2 * c + 2, :],
                        wql_t0[:, 2 * c:2 * c + 2, :],
                        XH[:, tch, 2 * c:2 * c + 2, :],
                        XL[:, tch, 2 * c:2 * c + 2, :],
                        first=(half == 0 and c == 0),
                        last=(half == 1 and c == CP - 1),
                        skip_hl=(c in (1, 3, 5, 7)))
            rope(ps, i0, 512, QT[:, 0, i0:i0 + 512])

    # ---------------- q-projection filler machinery ----------------
    filler = deque()

    wq_pref = {}

    def prefetch_wq(h):
        if h < HC and h not in wq_pref:
            wqh_t = w_pool.tile([128, CT, 128], FP8, tag="wh", bufs=4,
                                name="wqh_t")
            nc.sync.dma_start(wqh_t[:], wqh_d[h])
            wql_t = w_pool.tile([128, CT, 128], FP8, tag="wl", bufs=4,
                                name="wql_t")
            nc.sync.dma_start(wql_t[:], wql_d[h])
            wq_pref[h] = (wqh_t, wql_t)

    def push_qproj(h):
        """Queue head h's q-projection as small PE filler pieces."""
        prefetch_wq(h)
        state = {"w": wq_pref.pop(h)}

        for qc in range(4):
            for half in range(2):
                for quarter in range(4):
                    def mm_piece(qc=qc, half=half, quarter=quarter):
                        if half == 0 and quarter == 0:
                            state["ps"] = p_q.tile([128, 512], F32, tag="q",
                                                   bufs=1, name="ps_q")
                        ps = state["ps"]
                        wh, wl = state["w"]
                        off = half * 256
                        tch = qc * 2 + half
                        for c in range(quarter * 2, quarter * 2 + 2):
                            dr3(ps[:, off:off + 256],
                                wh[:, 2 * c:2 * c + 2, :],
                                wl[:, 2 * c:2 * c + 2, :],
                                XH[:, tch, 2 * c:2 * c + 2, :],
                                XL[:, tch, 2 * c:2 * c + 2, :],
                                first=(half == 0 and c == 0),
                                last=(half == 1 and c == CP - 1),
                                skip_hl=(c in (1, 3, 5, 7)))
                        if half == 1 and quarter == 3:
                            rope(state["ps"], qc * 512, 512,
                                 QT[:, h, qc * 512:qc * 512 + 512])
                    filler.append(mm_piece)

    # Fixed-position pacing: a qproj chunk is 8 pieces, each carrying
    # 6 DoubleRow matmuls (the first also opens the psum chunk, the
    # last also ropes); 14 drain at in-slot jts (one per jt, so the PE
    # never runs dry against ACT's exp cadence) and 2 at the slot
    # boundary so the PE has queued work while ACT computes the next
    # slot's first exp.
    PIECE_POINTS = (0, 1, 2, 3, 4, 5, 6, 7, 8, 9, 10, 11, 12, 13)

    def drain_at(point):
        k = 0
        while filler and k < PIECE_POINTS.count(point):
            filler.popleft()()
            k += 1

    def drain_boundary():
        k = 0
        while filler and k < 2:
            filler.popleft()()
            k += 1

    def flush_filler():
        while filler:
            filler.popleft()()

    # ---------------- attention slots ----------------
    wo_holder = []
    prefilled = set()
    ys_ev = [0]

    def oproj_group(key):
        tt, ob = key
        WOH, WOL = wo_holder[0]
        ps = p_q.tile([128, 512], F32, tag="q", bufs=1, name="ps_q")
        tb = tt * 128
        # hi-only (A) passes first so the group can start before the
        # wo_lo DMA lands.
        for half in range(2):
            off = half * 256
            o0 = ob * 512 + off
            for hp in range(HC // 2):
                nc.tensor.matmul(ps[:, off:off + 256],
                                 OTH[:, 2 * hp:2 * hp + 2, tb:tb + 128],
                                 WOH[:, 2 * hp:2 * hp + 2, o0:o0 + 256],
                                 start=(half == 0 and hp == 0), stop=False,
                                 perf_mode=DR)
        for half in range(2):
            off = half * 256
            o0 = ob * 512 + off
            for hp in range(HC // 2):
                nc.tensor.matmul(ps[:, off:off + 256],
                                 OTL[:, 2 * hp:2 * hp + 2, tb:tb + 128],
                                 WOH[:, 2 * hp:2 * hp + 2, o0:o0 + 256],
                                 start=False, stop=False, perf_mode=DR)
        for half in range(2):
            off = half * 256
            o0 = ob * 512 + off
            for hp in range(HC // 2):
                nc.tensor.matmul(ps[:, off:off + 256],
                                 OTH[:, 2 * hp:2 * hp + 2, tb:tb + 128],
                                 WOL[:, 2 * hp:2 * hp + 2, o0:o0 + 256],
                                 start=False,
                                 stop=(half == 1 and hp == HC // 2 - 1),
                                 perf_mode=DR)
        # y leaves scaled by 1/Y_SCALE and in bf16; the host multiplies
        # Y_SCALE into the partial-sum gather for free (bf16 also halves
        # the y DMA bytes).
        yt = st_pool.tile([128, 512], BF16, tag="yt8", bufs=2, name="yt")
        if ys_ev[0] % 2 == 0:
            nc.scalar.copy(yt[:], ps[:])
        else:
            nc.vector.tensor_copy(yt[:], ps[:])
        ys_ev[0] += 1
        nc.sync.dma_start(y_d[tt * 128:(tt + 1) * 128, ob * 512:ob * 512 + 512],
                          yt[:])
        prefilled.add(key)

    og_iter = iter([(tt, ob) for tt in range(3) for ob in range(4)][:11])
    with (
        tc.tile_pool(name="p_s", bufs=1, space="PSUM", side="right") as p_s,
        tc.tile_pool(name="p_u", bufs=1, space="PSUM", side="right") as p_u,
        tc.tile_pool(name="p_z", bufs=1, space="PSUM", side="right") as p_z,
    ):
        def make_slot(h, u):
            return {
                "h": h, "u": u, "g": h // 4, "i0": u * 1024,
                "U": p_u.tile([128, 1024], F32, tag="U", bufs=1, name="ps_U"),
                "Z": p_z.tile([128, 512], F32, tag="Z", bufs=1, name="ps_Z"),
                "Es": {}, "zred": [None, None],
            }

        def s_score(st, jt):
            ps = p_s.tile([128, 1024], F32, tag="S", bufs=2, name="ps_S")
            kt_sl = KT[:, st["g"], jt * 128:(jt + 1) * 128]
            i0 = st["i0"]
            nc.tensor.matmul(ps[:, 0:512], kt_sl,
                             QT[:, st["h"], i0:i0 + 512],
                             start=True, stop=True)
            nc.tensor.matmul(ps[:, 512:1024], kt_sl,
                             QT[:, st["h"], i0 + 512:i0 + 1024],
                             start=True, stop=True)
            E = e_pool.tile([128, 1024], BF16, tag="e", bufs=6, name="E")
            nc.scalar.activation(E[:], ps[:], Exp, scale=EXP_SCALE)
            st["Es"][jt] = E

        def s_av(st, jt):
            E = st["Es"][jt]
            v_sl = VV[:, jt, st["g"] * 128:(st["g"] + 1) * 128]
            s0, sp = (jt == 0), (jt == JT - 1)
            U_ps = st["U"]
            nc.tensor.matmul(U_ps[:, 0:512], v_sl, E[:, 0:512],
                             start=s0, stop=sp)
            nc.tensor.matmul(U_ps[:, 512:1024], v_sl, E[:, 512:1024],
                             start=s0, stop=sp)

        def s_z(st, jt):
            # Z column layout: [jt-half, ib, jt%8] so the first half's
            # reduce can run mid-slot and the next slot's writes never
            # collide with this slot's late reduce.
            E = st["Es"].pop(jt)
            Z_ps = st["Z"]
            col0 = (jt // 8) * 64 + (jt % 8)
            for ib in range(8):
                nc.tensor.matmul(
                    Z_ps[:, col0 + ib * 8:col0 + ib * 8 + 1],
                    E[:, ib * 128:(ib + 1) * 128], ones_t[:],
                    start=True, stop=True)

        def s_reduce_half(st, half):
            zr = rz_pool.tile([128, 8], F32, tag=f"zred{half}", bufs=1,
                              name="zred")
            nc.vector.tensor_reduce(
                zr[:],
                st["Z"][:, half * 64:half * 64 + 64].rearrange(
                    "p (a b) -> p a b", a=8),
                axis=AxX, op=Add)
            st["zred"][half] = zr

        def s_ztail(st):
            h, i0, U_ps, Z_ps = st["h"], st["i0"], st["U"], st["Z"]
            # Evict U to SBUF right away so the U psum bank is free
            # before the next slot's first AV matmul.
            # softmax denominator first on DVE (short ops) so the PE
            # transpose is unblocked before the 1.2us usb eviction runs.
            s_reduce_half(st, 1)
            zs = rz_pool.tile([128, 8], F32, tag="zs", bufs=1, name="zs")
            nc.vector.tensor_add(zs[:], st["zred"][0][:], st["zred"][1][:])
            rz = rz_pool.tile([128, 8], F32, tag="rz", bufs=1, name="rz")
            nc.vector.reciprocal(rz[:], zs[:])
            usb = rz_pool.tile([128, 1024], BF16, tag="usb", bufs=1,
                               name="usb")
            nc.vector.tensor_copy(usb[:], U_ps[:])
            nc.tensor.matmul(Z_ps[0:8, 256:384], rz[:], ident_t[:],
                             start=True, stop=True, is_transpose=True)
            rzt = rz_pool.tile([8, 128], BF16, tag="rzt", bufs=1, name="rzt")
            nc.vector.tensor_copy(rzt[:], Z_ps[0:8, 256:384])
            # flatten [8,128] -> [1,1024] via a DRAM round trip (plain
            # DMAs, off the critical path), then one gpsimd broadcast.
            # (hw partition_broadcast requires base partition 0, so the
            # 8-direct-broadcast variant fails BIR verification.)
            rsc = dram_pool.tile([8, 128], BF16, tag="rsc", bufs=2,
                                 name="rsc")
            nc.sync.dma_start(rsc[:], rzt[:])
            rzrow = rz_pool.tile([1, 1024], BF16, tag="rzrow", bufs=1,
                                 name="rzrow")
            nc.sync.dma_start(
                rzrow[:], rsc.rearrange("a b -> (a b)").unsqueeze(0))
            rzf = rz_pool.tile([128, 1024], BF16, tag="rzf", bufs=1,
                               name="rzf")
            nc.gpsimd.partition_broadcast(rzf[:], rzrow[0:1, :])
            tmp = rz_pool.tile([128, 1024], BF16, tag="ottmp", bufs=2,
                               name="ottmp")
            nc.gpsimd.tensor_mul(tmp[:], usb[:], rzf[:])
            nc.gpsimd.tensor_copy(OTH[:, h, i0:i0 + 1024], tmp[:])
            # Keep the lo-residual subtract on Pool: DVE executes in
            # order, and parking a late-resolving op there head-of-line
            # blocks the next slot's zred/usb chain (PE stalls on the
            # Z/U psum banks).
            nc.gpsimd.tensor_sub(OTL[:, h, i0:i0 + 1024], tmp[:],
                                 OTH[:, h, i0:i0 + 1024])

        # u-major slot order: all heads' first query half completes by
        # slot 7, so o_proj prefill groups (which only touch tt<8 token
        # blocks) stop depending on the LAST slot's late Pool chain.
        slots = [(h, u) for u in range(2) for h in range(HC)]
        cur = make_slot(0, 0)
        s_score(cur, 0)
        s_score(cur, 1)
        pushed = {0}
        for idx, (h, u) in enumerate(slots):
            if u == 0:
                prefetch_wq(h + 2)
                # supply one head ahead of the drain schedule so early
                # slots have filler slack (a rope-delayed chunk can't
                # starve the piece queue).
                for hh in ((1, 2) if h == 0 else (h + 2,)):
                    if hh < HC and hh not in pushed:
                        pushed.add(hh)
                        push_qproj(hh)
            st = cur
            nxt = None
            for jt in range(JT):
                s_av(st, jt)
                s_z(st, jt)
                if jt + 2 < JT:
                    s_score(st, jt + 2)
                elif idx + 1 < len(slots):
                    # Deep slot overlap: the next slot's first two scores
                    # go out at jt 14/15, reusing the psum bufs freed by
                    # exp(12)/exp(13), so exp_n(0) is already done when
                    # the next slot's first AV issues -- no boundary gap.
                    if nxt is None:
                        nxt = make_slot(*slots[idx + 1])
                    s_score(nxt, jt + 2 - JT)
                if jt == 8:
                    s_reduce_half(st, 0)
                if jt in PIECE_POINTS:
                    drain_at(jt)
                if ((idx == 14 and jt in (11, 13, 15))
                        or (idx == 15 and jt in (2, 5, 8, 11, 14))):
                    # the filler is exhausted by slot 13; fill the last
                    # two slots' ACT-paced slack with o_proj groups.
                    # slot 14's groups sit late enough that the wo DMAs
                    # (issued at slot 13's end) have landed.
                    oproj_group(next(og_iter))
            if idx + 1 < len(slots):
                cur = nxt
                if filler:
                    drain_boundary()
                elif idx >= 14:
                    oproj_group(next(og_iter))
            else:
                # keep the PE busy while the final denominator chain
                # (zred/zs/rz on DVE) runs, so the ztail transpose that
                # precedes the o_proj phase in the in-order PE stream
                # doesn't stall it.
                oproj_group(next(og_iter))
                oproj_group(next(og_iter))
            s_ztail(st)
            if idx == 13:
                # X no longer needed (last q-projection drained); swap the
                # X heap space for the wo hi/lo tiles. wo_hi first: the
                # prefill groups' A-passes only need it + OT.
                flush_filler()
                x_ctx.close()
                wo_pool = ctx.enter_context(
                    tc.tile_pool(name="wo", bufs=1, side="left"))
                WOH = wo_pool.tile([128, HC, S], FP8, tag="woh", bufs=1,
                                   name="woh_t")
                WOL = wo_pool.tile([128, HC, S], FP8, tag="wol", bufs=1,
                                   name="wol_t")
                wo_holder.append((WOH, WOL))
                # one big strided DMA per tensor: same bytes, but 2 HWDGE
                # generations instead of 16 (~4us less queue time), so the
                # slot-14 o_proj prefill groups unblock earlier.
                nc.sync.dma_start(WOH[:], woh_d.rearrange("h p s -> p h s"))
                wol_r = wol_d.rearrange("h p s -> p h s")
                nc.sync.dma_start(WOL[:, 0:HC // 2, :], wol_r[:, 0:HC // 2, :])
                nc.sync.dma_start(WOL[:, HC // 2:, :], wol_r[:, HC // 2:, :])

    pq_ctx.close()

    # ---------------- output projection ----------------
    WOH, WOL = wo_holder[0]
    with (
        tc.tile_pool(name="p_y", bufs=1, space="PSUM", side="right") as p_y,
        tc.tile_pool(name="ys", bufs=1, side="right") as ys_pool,
    ):
        ev = 0
        remaining = [(tt, ob) for tt in range(16) for ob in range(4)
                     if (tt, ob) not in prefilled]
        for gi, (tt, ob) in enumerate(remaining):
            tb = tt * 128
            if gi == len(remaining) - 1:
                # final group: two independent psum half-tiles, each
                # evicted + DMA'd as soon as its matmuls finish, so the
                # post-PE drain is one small eviction + DMA.
                for half in range(2):
                    o0 = ob * 512 + half * 256
                    psh = p_y.tile([128, 256], F32, tag="yl", bufs=2,
                                   name="ps_yl")
                    for hp in range(HC // 2):
                        dr3(psh[:],
                            OTH[:, 2 * hp:2 * hp + 2, tb:tb + 128],
                            OTL[:, 2 * hp:2 * hp + 2, tb:tb + 128],
                            WOH[:, 2 * hp:2 * hp + 2, o0:o0 + 256],
                            WOL[:, 2 * hp:2 * hp + 2, o0:o0 + 256],
                            first=(hp == 0), last=(hp == HC // 2 - 1))
                    yth = ys_pool.tile([128, 256], BF16, tag="ytl", bufs=2,
                                       name="ytl")
                    if half == 0:
                        nc.scalar.copy(yth[:], psh[:])
                        nc.scalar.dma_start(y_d[tb:tb + 128, o0:o0 + 256],
                                            yth[:])
                    else:
                        nc.vector.tensor_copy(yth[:], psh[:])
                        nc.sync.dma_start(y_d[tb:tb + 128, o0:o0 + 256],
                                          yth[:])
                continue
            ps = p_y.tile([128, 512], F32, tag="y", bufs=6, name="ps_y")
            for half in range(2):
                off = half * 256
                o0 = ob * 512 + off
                for hp in range(HC // 2):
                    dr3(ps[:, off:off + 256],
                        OTH[:, 2 * hp:2 * hp + 2, tb:tb + 128],
                        OTL[:, 2 * hp:2 * hp + 2, tb:tb + 128],
                        WOH[:, 2 * hp:2 * hp + 2, o0:o0 + 256],
                        WOL[:, 2 * hp:2 * hp + 2, o0:o0 + 256],
                        first=(half == 0 and hp == 0),
                        last=(half == 1 and hp == HC // 2 - 1))
            yt = ys_pool.tile([128, 512], BF16, tag="yt", bufs=8,
                              name="yt")
            if ev % 2 == 0:
                nc.scalar.copy(yt[:], ps[:])
            else:
                nc.vector.tensor_copy(yt[:], ps[:])
            ev += 1
            nc.sync.dma_start(
                y_d[tt * 128:(tt + 1) * 128, ob * 512:ob * 512 + 512],
                yt[:])

    ctx.close()


def _build(repeat=1):
    import concourse.mybir as mybir
    import concourse.tile as tile
    from concourse import bacc

    F32 = mybir.dt.float32
    BF16 = mybir.dt.bfloat16
    FP8 = mybir.dt.float8e4

    nc = bacc.Bacc("TRN2", target_bir_lowering=False, debug=False)
    # X chunked [tch, 128, CT, 256] so each 256-token chunk DMA has 4KB
    # contiguous per-partition runs (sub-512B descriptors pay 2x on the
    # DMA bus).
    xh_d = nc.dram_tensor("xh", [8, 128, CT, 256], FP8,
                          kind="ExternalInput").ap()
    xl_d = nc.dram_tensor("xl", [8, 128, CT, 256], FP8,
                          kind="ExternalInput").ap()
    cosk_d = nc.dram_tensor("cosk", [128, S], BF16, kind="ExternalInput").ap()
    sinkm_d = nc.dram_tensor("sinkm", [128, S], BF16,
                             kind="ExternalInput").ap()
    wqh_d = nc.dram_tensor("wqh", [HC, 128, CT, 128], FP8,
                           kind="ExternalInput").ap()
    wql_d = nc.dram_tensor("wql", [HC, 128, CT, 128], FP8,
                           kind="ExternalInput").ap()
    wkh_d = nc.dram_tensor("wkh", [KVC, 128, CT, 128], FP8,
                           kind="ExternalInput").ap()
    wkl_d = nc.dram_tensor("wkl", [KVC, 128, CT, 128], FP8,
                           kind="ExternalInput").ap()
    wvh_d = nc.dram_tensor("wvh", [128, CT, 256], FP8,
                           kind="ExternalInput").ap()
    wvl_d = nc.dram_tensor("wvl", [128, CT, 256], FP8,
                           kind="ExternalInput").ap()
    woh_d = nc.dram_tensor("woh", [HC, 128, S], FP8, kind="ExternalInput").ap()
    wol_d = nc.dram_tensor("wol", [HC, 128, S], FP8, kind="ExternalInput").ap()
    ones_d = nc.dram_tensor("ones", [128, 1], BF16, kind="ExternalInput").ap()
    ident_d = nc.dram_tensor("ident", [128, 128], F32,
                             kind="ExternalInput").ap()
    y_d = nc.dram_tensor("y", [S, HID], BF16, kind="ExternalOutput").ap()

    with tile.TileContext(nc) as tc:
        for _ in range(repeat):
            _emit(nc, tc, (xh_d, xl_d, cosk_d, sinkm_d, wqh_d, wql_d, wkh_d,
                           wkl_d, wvh_d, wvl_d, woh_d, wol_d, ones_d, ident_d,
                           y_d))
    nc.compile()
    return nc


class _Runner:
    """Persistent-jit PJRT executor (axon) / NRT executor (native)."""

    def __init__(self, nc):
        self.nc = nc
        from concourse._compat import axon_active
        self.axon = axon_active()
        if not self.axon:
            return
        import jax
        from jax.sharding import Mesh, PartitionSpec
        from jax.experimental.shard_map import shard_map
        import concourse.mybir as mybir
        from concourse.bass2jax import (
            _bass_exec_p, install_neuronx_cc_hook, partition_id_tensor)

        install_neuronx_cc_hook()
        partition_name = (nc.partition_id_tensor.name
                          if nc.partition_id_tensor else None)
        in_names, out_names, out_avals, zero_outs = [], [], [], []
        for alloc in nc.m.functions[0].allocations:
            if not isinstance(alloc, mybir.MemoryLocationSet):
                continue
            name = alloc.memorylocations[0].name
            if alloc.kind == "ExternalInput":
                if name != partition_name:
                    in_names.append(name)
            elif alloc.kind == "ExternalOutput":
                shape = tuple(alloc.tensor_shape)
                dtype = mybir.dt.np(alloc.dtype)
                out_names.append(name)
                out_avals.append(jax.core.ShapedArray(shape, dtype))
                zero_outs.append(np.zeros(shape, dtype))
        self.in_names, self.out_names = in_names, out_names
        self.zero_outs = zero_outs
        n_params, n_outs = len(in_names), len(out_names)
        all_in = in_names + out_names
        if partition_name is not None:
            all_in.append(partition_name)
        donate = tuple(range(n_params, n_params + n_outs))

        def _body(*args):
            operands = list(args)
            if partition_name is not None:
                operands.append(partition_id_tensor())
            return tuple(_bass_exec_p.bind(
                *operands,
                out_avals=tuple(out_avals),
                in_names=tuple(all_in),
                out_names=tuple(out_names),
                lowering_input_output_aliases=(),
                sim_require_finite=True,
                sim_require_nnan=True,
                nc=nc,
            ))

        devices = jax.devices()[:N_CORES]
        mesh = Mesh(np.asarray(devices), ("core",))
        self._fn = jax.jit(
            shard_map(_body, mesh=mesh,
                      in_specs=(PartitionSpec("core"),) * (n_params + n_outs),
                      out_specs=(PartitionSpec("core"),) * n_outs,
                      check_rep=False),
            donate_argnums=donate, keep_unused=True,
        )

    def run(self, in_maps):
        if not self.axon:
            from concourse import bass_utils
            res = bass_utils.run_bass_kernel_spmd(
                self.nc, in_maps, core_ids=list(range(N_CORES)))
            return res.results
        concat_in = [
            np.concatenate([np.asarray(in_maps[c][n]) for c in range(N_CORES)],
                           axis=0)
            for n in self.in_names
        ] + [np.concatenate([z] * N_CORES, axis=0) for z in self.zero_outs]
        outs = [np.asarray(o) for o in self._fn(*concat_in)]
        per_core = []
        for c in range(N_CORES):
            d = {}
            for name, o in zip(self.out_names, outs):
                rows = o.shape[0] // N_CORES
                d[name] = o[c * rows:(c + 1) * rows]
            per_core.append(d)
        return per_core


def _prep_inputs(x, cos, sin, wq, wk, wv, wo):
    import concourse.mybir as mybir
    f32 = np.float32
    bf16 = mybir.dt.np(mybir.dt.bfloat16)
    fp8 = mybir.dt.np(mybir.dt.float8e4)

    def split8(arr, scale):
        s = np.ascontiguousarray(arr, f32) * f32(scale)
        hi = s.astype(fp8)
        lo = (s - hi.astype(f32)).astype(fp8)
        return hi, lo

    cosT = np.asarray(cos).T.astype(f32)    # [128, S]
    # half-swapped sin table: row r holds the sin factor that multiplies
    # q[row r] when producing the OTHER half's rotate_half term, so both
    # tensor_mul inputs share a base partition (hw BIR constraint).
    sinT = np.asarray(sin).T.astype(f32)
    sinm = np.concatenate([sinT[64:128], -sinT[0:64]], axis=0).copy()
    cosT = np.ascontiguousarray(cosT).astype(bf16)
    sinm = np.ascontiguousarray(sinm).astype(bf16)
    ones = np.full((128, 1), 1.0 / EO, bf16)
    ident = np.eye(128, dtype=f32)
    x = np.asarray(x, f32)
    wq = np.asarray(wq, f32)
    wk = np.asarray(wk, f32)
    wv = np.asarray(wv, f32)
    wo = np.asarray(wo, f32)

    in_maps = []
    for c in range(N_CORES):
        b, kh = c // 2, c % 2
        # X^T packed [tch, p, ct, tok%256]: [p, ct, j] = x[b, j, ct*128+p]
        xt = x[b].T.reshape(CT, 128, S).transpose(1, 0, 2)
        xt = xt.reshape(128, CT, 8, 256).transpose(2, 0, 1, 3)
        xh, xl = split8(xt, AX)
        # wq rows for this core's heads -> [h, p(ct-part), ct, c(col)]
        wq_c = wq[kh * 1024:(kh + 1) * 1024, :]
        wqt = wq_c.reshape(HC, 128, CT, 128).transpose(0, 3, 2, 1)
        wqh, wql = split8(wqt, BW)
        wk_c = wk[kh * 256:(kh + 1) * 256, :]
        wkt = wk_c.reshape(KVC, 128, CT, 128).transpose(0, 3, 2, 1)
        wkh, wkl = split8(wkt, BW)
        wv_c = wv[kh * 256:(kh + 1) * 256, :]
        wvt = wv_c.reshape(256, CT, 128).transpose(2, 1, 0)
        wvh, wvl = split8(wvt, BW)
        # wo columns for this core's heads -> [h, p(=d), out]
        wot = wo[:, kh * 1024:(kh + 1) * 1024].T.reshape(HC, 128, S)
        woh, wol = split8(wot, GW)
        in_maps.append({
            "xh": xh, "xl": xl, "cosk": cosT, "sinkm": sinm,
            "wqh": wqh, "wql": wql, "wkh": wkh, "wkl": wkl,
            "wvh": wvh, "wvl": wvl, "woh": woh, "wol": wol,
            "ones": ones, "ident": ident,
        })
    return in_maps


def kernel(x, cos, sin, wq, wk, wv, wo):
    if "nc" not in _cache:
        _cache["nc"] = _build()
        _cache["runner"] = _Runner(_cache["nc"])
    runner = _cache["runner"]
    in_maps = _prep_inputs(x, cos, sin, wq, wk, wv, wo)
    results = runner.run(in_maps)
    y = np.empty((B, S, HID), np.float32)
    for b in range(B):
        y[b] = (results[2 * b]["y"].astype(np.float32)
                + results[2 * b + 1]["y"].astype(np.float32)) * np.float32(
                    Y_SCALE)
    return y


# revision 58
# speedup vs baseline: 1.0014x; 1.0014x over previous
"""Trainium2 Bass kernel for nn_Attention_55130200211640 (v3).

GQA attention block: q/k/v projections + RoPE (theta=1e6) + non-causal
softmax attention (16 q-heads, 4 kv-heads, head_dim 128) + output
projection. B=4, S=2048, HID=2048, fp32 I/O.

Sharding: (batch x 4) x (kv-group-half x 2) = 8 cores, tensor-parallel
over heads within a batch. Each core computes, for its batch, the full
2048-token sequence restricted to 2 of the 4 kv groups (= 8 of the 16
q heads): q/k/v projections, RoPE, attention, and a PARTIAL o_proj
(contraction over its 8 heads only). The host adds the two partial y's
per batch.

v3: all four projections (q/k/v/o) run as fp8e4 DoubleRow matmuls with
hi+lo operand splitting. Each operand t is stored as t_hi = fp8(s*t),
t_lo = fp8(s*t - t_hi); one DoubleRow pass computes two 128-contraction
products at 0.5 cycles/out-col, and per k-tile-pair the three passes
  (w1_hi,w2_hi)x(x1_hi,x2_hi) + (w1_lo,w2_lo)x(x1_hi,x2_hi)
                              + (w1_hi,w2_hi)x(x1_lo,x2_lo)
reconstruct w.x to ~bf16 accuracy (lo.lo dropped) at 0.75x the bf16
column cost. The q-projection additionally drops the x_lo pass on half
its k-tile pairs (skip_hl): +1.1e-2 deterministic L2 error (measured
total 1.70e-2 vs the 2e-2 gate) for 1/6 of the q-proj PE time.
Scores and AV stay bf16 (E/K/Q splits don't pay there).
Scale bookkeeping: x scaled by AX, wq/wk/wv by BW -> exp scale folds
1/(AX^2 BW^2); V path carries AX*BW; OT split scaled by EO (folded as
1/EO into the ones vector so rz = EO/Z); wo scaled by GW; final y
eviction multiplies by 1/(AX*BW*EO*GW).

Per-core dataflow ("contraction-on-partition" layouts everywhere):
  preamble: X^T hi/lo fp8 resident (chunk-major for contiguous DMA);
            K^T [d,2|S] and V [j,256] computed (DoubleRow) + roped,
            Q^T head 0.
  slots:    16 slots in U-MAJOR order: all 8 heads' first query half
            (u=0), then all second halves -- so every head's u0 OT is
            done by slot 7 and o_proj prefill never waits on the last
            slot's normalize chain. Per slot: S^T[j,i] = K^T_g . Q^T_h
            on PE (bf16) -> exp on ACT (scale folded) -> E bf16;
            U^T[d,i] = sum_j V E (psum-accumulated, bf16); Z[i] =
            sum_j E via stationary-E matmuls with a [128,1]
            (1/EO)-valued moving operand; Z -> free-axis reduce ->
            recip -> PE transpose -> DRAM-roundtrip flatten -> gpsimd
            broadcast -> tmp = U*(EO/Z) on gpsimd -> OT_hi fp8 (gpsimd
            cast) + OT_lo fp8 (gpsimd subtract; keeping it on Pool
            avoids head-of-line blocking DVE's zred/usb chain). The
            next head's q-projection DoubleRow passes are interleaved
            as PE filler (8 pieces/chunk, one per jt) plus two at the
            slot boundary; the next slot's first two scores are emitted
            at jt 14/15 so its first AV finds its exp done. Slots 14/15
            (no filler left) interleave o_proj groups instead.
  o_proj:   y[i,o] = sum_h OT_h . wo_h via DoubleRow over head pairs,
            psum-accumulated, evicted as bf16 plain copies (the unscale
            factor is folded into the host-side partial-sum gather);
            the final group evicts per 256-half to shorten the drain.
"""

import numpy as np

B, S, HID = 4, 2048, 2048
H, KV, D = 16, 4, 128
N_CORES = 8
HC = 8                 # heads per core
KVC = 2                # kv groups per core
CT = HID // 128        # contraction tiles
CP = CT // 2           # contraction tile pairs (DoubleRow)
JT = S // 128          # key tiles
SCALE = 1.0 / float(np.sqrt(D))

AX = 16.0              # x hi/lo split scale
BW = 800.0             # wq/wk/wv split scale
EO = 1.0 / 320.0       # OT split scale (1/EO exact in bf16)
GW = 800.0             # wo split scale
EXP_SCALE = SCALE / (AX * AX * BW * BW)
Y_SCALE = 1.0 / (AX * BW * EO * GW)

_cache = {}


def _emit(nc, tc, io):
    import concourse.mybir as mybir
    from collections import deque
    from contextlib import ExitStack

    F32 = mybir.dt.float32
    BF16 = mybir.dt.bfloat16
    FP8 = mybir.dt.float8e4
    DR = mybir.MatmulPerfMode.DoubleRow
    Exp = mybir.ActivationFunctionType.Exp
    Copy = mybir.ActivationFunctionType.Copy
    AxX = mybir.AxisListType.X
    Add = mybir.AluOpType.add

    (xh_d, xl_d, cosk_d, sinkm_d, wqh_d, wql_d, wkh_d, wkl_d, wvh_d, wvl_d,
     woh_d, wol_d, ones_d, ident_d, y_d) = io

    ctx = ExitStack()

    # ---------------- persistent SBUF tiles (left heap) ----------------
    const_pool = ctx.enter_context(tc.tile_pool(name="const", bufs=1, side="left"))
    ones_t = const_pool.tile([128, 1], BF16)
    ident_t = const_pool.tile([128, 128], F32)
    COS = const_pool.tile([128, S], BF16)
    SINM = const_pool.tile([128, S], BF16)

    dram_pool = ctx.enter_context(tc.tile_pool(name="drp", bufs=1, space="DRAM"))
    kv_pool = ctx.enter_context(tc.tile_pool(name="kv", bufs=1, side="left"))
    KT = kv_pool.tile([128, KVC, S], BF16)        # [d, g, j]
    VV = kv_pool.tile([128, JT, KVC * 128], BF16)  # [j, jt, g*128+d]
    q_pool = ctx.enter_context(tc.tile_pool(name="qt", bufs=1, side="left"))
    QT = q_pool.tile([128, HC, S], BF16)          # [d, h, i]
    o_pool = ctx.enter_context(tc.tile_pool(name="ot", bufs=1, side="left"))
    OTH = o_pool.tile([128, HC, S], FP8)          # [d, h, i] hi
    OTL = o_pool.tile([128, HC, S], FP8)          # [d, h, i] lo

    # X^T hi/lo resident for the whole projection span. Opened last on
    # the left heap so it can be closed (LIFO) mid-emission to make room
    # for the wo_lo tile.
    x_ctx = ExitStack()
    x_pool = x_ctx.enter_context(tc.tile_pool(name="xp", bufs=1, side="left"))
    # chunk-major so each 256-token chunk DMA is contiguous on both
    # sides (4KB descriptors; sub-512B descriptors pay 2x on the bus).
    XH = x_pool.tile([128, 8, CT, 256], FP8)      # [hid%128, tch, ct, tok%256]
    XL = x_pool.tile([128, 8, CT, 256], FP8)

    w_pool = ctx.enter_context(tc.tile_pool(name="wp", bufs=2, side="right"))
    st_pool = ctx.enter_context(tc.tile_pool(name="st", bufs=1, side="right"))
    e_pool = ctx.enter_context(tc.tile_pool(name="ep", bufs=3, side="right"))
    rz_pool = ctx.enter_context(tc.tile_pool(name="rz", bufs=1, side="right"))

    # Persistent psum pool for projection chunks: 1 bank, used by the
    # q-projection filler during the slots as well.
    pq_ctx = ExitStack()
    p_q = pq_ctx.enter_context(
        tc.tile_pool(name="p_q", bufs=1, space="PSUM", side="left"))

    def rope(ps, c0, n, dst):
        """RoPE a [128, n] psum tile (layout [d, pos], positions c0:c0+n)
        -> bf16 SBUF dst. rotate_half is a cross-partition half-swap; the
        sign lives in SINM (rows 0:64 pre-negated on the host).
        The psum is evicted to bf16 SBUF in ONE copy (so the projection
        psum bank frees ~1.4us earlier for the next chunk) and the rope
        arithmetic runs all-bf16, which qualifies for the DVE 2x mode
        (327ns vs 658ns per [128,512] op)."""
        qsb = st_pool.tile([128, 512], BF16, tag="qsb", bufs=2, name="qsb")
        nc.any.tensor_copy(qsb[:, 0:n], ps[:])
        rope_ops(qsb, c0, n, dst)

    def rope_ops(qsb, c0, n, dst):
        tmp = st_pool.tile([128, 512], BF16, tag="tmp", bufs=2, name="tmp")
        stage = st_pool.tile([128, 512], BF16, tag="stage", bufs=2,
                             name="stage")
        # SINM halves are pre-swapped on the host so each mul reads both
        # SBUF inputs at the SAME base partition (hw BIR rule); only the
        # OUTPUT partitions are swapped, as in the proven psum variant.
        nc.vector.tensor_mul(stage[0:64, 0:n], qsb[64:128, 0:n],
                             SINM[64:128, c0:c0 + n])
        nc.vector.tensor_mul(stage[64:128, 0:n], qsb[0:64, 0:n],
                             SINM[0:64, c0:c0 + n])
        nc.vector.tensor_mul(tmp[:, 0:n], qsb[:, 0:n], COS[:, c0:c0 + n])
        nc.vector.tensor_add(dst, stage[:, 0:n], tmp[:, 0:n])

    def dr3(ps_sl, wh, wl, xh, xl, first, last, skip_hl=False):
        """Emit the 3 DoubleRow passes for one k-tile pair: hh + lh + hl.
        wh/wl: stationary [128, 2, M]; xh/xl: moving [128, 2, N<=256].
        skip_hl drops the x_lo correction for this pair -- used on 2 of
        8 q-proj pairs to trade ~1.3% extra (deterministic, measured)
        error for 1/12 of the q-projection's PE time."""
        nc.tensor.matmul(ps_sl, wh, xh, start=first, stop=False, perf_mode=DR)
        if skip_hl:
            nc.tensor.matmul(ps_sl, wl, xh, start=False, stop=last,
                             perf_mode=DR)
            return
        nc.tensor.matmul(ps_sl, wl, xh, start=False, stop=False, perf_mode=DR)
        nc.tensor.matmul(ps_sl, wh, xl, start=False, stop=last, perf_mode=DR)

    # ---------------- preamble: K, V, Q(0) projections ----------------
    # DMA priority: the hi tensors for the first chunk's A-passes land
    # first, then the lo tensors, then the X stream stays ahead of the
    # 256-token chunk walk (~5.1us compute / ~3.2us DMA per chunk).
    wkh_ts, wkl_ts = [], []
    for g in range(KVC):
        wkh_t = w_pool.tile([128, CT, 128], FP8, tag="wh", bufs=4,
                            name="wkh_t")
        nc.sync.dma_start(wkh_t[:], wkh_d[g])
        wkh_ts.append(wkh_t)
    nc.sync.dma_start(XH[:, 0], xh_d[0])
    nc.sync.dma_start(XL[:, 0], xl_d[0])
    for g in range(KVC):
        wkl_t = w_pool.tile([128, CT, 128], FP8, tag="wl", bufs=4, name="wkl_t")
        nc.sync.dma_start(wkl_t[:], wkl_d[g])
        wkl_ts.append(wkl_t)
    wvh_t = w_pool.tile([128, CT, 256], FP8, tag="wvh", bufs=1, name="wvh_t")
    nc.sync.dma_start(wvh_t[:], wvh_d[:])
    nc.scalar.dma_start(COS[:, 0:1024], cosk_d[:, 0:1024])
    nc.scalar.dma_start(SINM[:, 0:1024], sinkm_d[:, 0:1024])
    wvl_t = w_pool.tile([128, CT, 256], FP8, tag="wvl", bufs=1, name="wvl_t")
    nc.sync.dma_start(wvl_t[:], wvl_d[:])
    for tch in range(1, 8):
        t0 = tch * 256
        nc.sync.dma_start(XH[:, tch], xh_d[tch])
        nc.sync.dma_start(XL[:, tch], xl_d[tch])
        if tch == 1:
            nc.scalar.dma_start(COS[:, 1024:2048], cosk_d[:, 1024:2048])
            nc.scalar.dma_start(SINM[:, 1024:2048], sinkm_d[:, 1024:2048])
    wqh_t0 = w_pool.tile([128, CT, 128], FP8, tag="wh", bufs=4, name="wqh_t")
    nc.sync.dma_start(wqh_t0[:], wqh_d[0])
    wql_t0 = w_pool.tile([128, CT, 128], FP8, tag="wl", bufs=4, name="wql_t")
    nc.sync.dma_start(wql_t0[:], wql_d[0])
    nc.sync.dma_start(ones_t[:], ones_d[:])
    nc.sync.dma_start(ident_t[:], ident_d[:])

    with tc.tile_pool(name="p_pre", bufs=1, space="PSUM", side="right") as p_pre:
        for tch in range(8):
            j0 = tch * 256
            kps = []
            for g in range(KVC):
                kps.append(p_pre.tile([128, 512], F32, tag="c", bufs=6,
                                      name="ps_pre"))
            vps = []
            for jt in range(tch * 2, tch * 2 + 2):
                vps.append(p_pre.tile([128, 512], F32, tag="c", bufs=6,
                                      name="ps_pre"))
            # Pass order follows first-chunk DMA arrival: K-hi passes,
            # then XL-dependent K passes, then wkl passes + ropes, then V.
            for g in range(KVC):
                for c in range(CP):
                    nc.tensor.matmul(kps[g][:, 0:256],
                                     wkh_ts[g][:, 2 * c:2 * c + 2, :],
                                     XH[:, tch, 2 * c:2 * c + 2, :],
                                     start=(c == 0), stop=False, perf_mode=DR)
            for g in range(KVC):
                for c in range(CP):
                    nc.tensor.matmul(kps[g][:, 0:256],
                                     wkh_ts[g][:, 2 * c:2 * c + 2, :],
                                     XL[:, tch, 2 * c:2 * c + 2, :],
                                     start=False, stop=False, perf_mode=DR)
            for g in range(KVC):
                for c in range(CP):
                    nc.tensor.matmul(kps[g][:, 0:256],
                                     wkl_ts[g][:, 2 * c:2 * c + 2, :],
                                     XH[:, tch, 2 * c:2 * c + 2, :],
                                     start=False, stop=(c == CP - 1),
                                     perf_mode=DR)
                rope(kps[g][:, 0:256], j0, 256, KT[:, g, j0:j0 + 256])
            for i, jt in enumerate(range(tch * 2, tch * 2 + 2)):
                jb = (jt % 2) * 128
                for c in range(CP):
                    nc.tensor.matmul(vps[i][:, 0:256],
                                     XH[:, tch, 2 * c:2 * c + 2, jb:jb + 128],
                                     wvh_t[:, 2 * c:2 * c + 2, :],
                                     start=(c == 0), stop=False, perf_mode=DR)
                for c in range(CP):
                    nc.tensor.matmul(vps[i][:, 0:256],
                                     XL[:, tch, 2 * c:2 * c + 2, jb:jb + 128],
                                     wvh_t[:, 2 * c:2 * c + 2, :],
                                     start=False, stop=False, perf_mode=DR)
                for c in range(CP):
                    nc.tensor.matmul(vps[i][:, 0:256],
                                     XH[:, tch, 2 * c:2 * c + 2, jb:jb + 128],
                                     wvl_t[:, 2 * c:2 * c + 2, :],
                                     start=False, stop=(c == CP - 1),
                                     perf_mode=DR)
                nc.scalar.copy(VV[:, jt, :], vps[i][:, 0:256])
        # Q projection head 0.
        for qc in range(4):
            i0 = qc * 512
            ps = p_pre.tile([128, 512], F32, tag="c", bufs=6, name="ps_pre")
            for half in range(2):
                off = half * 256
                tch = qc * 2 + half
                for c in range(CP):
                    dr3(ps[:, off:off + 256],
                        wqh_t0[:, 2 * c:2 * c + 2, :],
                        wql_t0[:, 2 * c:2 * c + 2, :],
                        XH[:, tch, 2 * c:2 * c + 2, :],
                        XL[:, tch, 2 * c:2 * c + 2, :],
                        first=(half == 0 and c == 0),
                        last=(half == 1 and c == CP - 1),
                        skip_hl=(c in (1, 3, 5, 7)))
            rope(ps, i0, 512, QT[:, 0, i0:i0 + 512])

    # ---------------- q-projection filler machinery ----------------
    filler = deque()

    wq_pref = {}

    def prefetch_wq(h):
        if h < HC and h not in wq_pref:
            wqh_t = w_pool.tile([128, CT, 128], FP8, tag="wh", bufs=4,
                                name="wqh_t")
            nc.sync.dma_start(wqh_t[:], wqh_d[h])
            wql_t = w_pool.tile([128, CT, 128], FP8, tag="wl", bufs=4,
                                name="wql_t")
            nc.sync.dma_start(wql_t[:], wql_d[h])
            wq_pref[h] = (wqh_t, wql_t)

    def push_qproj(h):
        """Queue head h's q-projection as small PE filler pieces."""
        prefetch_wq(h)
        state = {"w": wq_pref.pop(h)}

        for qc in range(4):
            for half in range(2):
                for quarter in range(4):
                    def mm_piece(qc=qc, half=half, quarter=quarter):
                        if half == 0 and quarter == 0:
                            state["ps"] = p_q.tile([128, 512], F32, tag="q",
                                                   bufs=1, name="ps_q")
                        ps = state["ps"]
                        wh, wl = state["w"]
                        off = half * 256
                        tch = qc * 2 + half
                        for c in range(quarter * 2, quarter * 2 + 2):
                            dr3(ps[:, off:off + 256],
                                wh[:, 2 * c:2 * c + 2, :],
                                wl[:, 2 * c:2 * c + 2, :],
                                XH[:, tch, 2 * c:2 * c + 2, :],
                                XL[:, tch, 2 * c:2 * c + 2, :],
                                first=(half == 0 and c == 0),
                                last=(half == 1 and c == CP - 1),
                                skip_hl=(c in (1, 3, 5, 7)))
                        if half == 1 and quarter == 3:
                            rope(state["ps"], qc * 512, 512,
                                 QT[:, h, qc * 512:qc * 512 + 512])
                    filler.append(mm_piece)

    # Fixed-position pacing: a qproj chunk is 8 pieces, each carrying
    # 6 DoubleRow matmuls (the first also opens the psum chunk, the
    # last also ropes); 14 drain at in-slot jts (one per jt, so the PE
    # never runs dry against ACT's exp cadence) and 2 at the slot
    # boundary so the PE has queued work while ACT computes the next
    # slot's first exp.
    PIECE_POINTS = (0, 1, 2, 3, 4, 5, 6, 7, 8, 9, 10, 11, 12, 13)

    def drain_at(point):
        k = 0
        while filler and k < PIECE_POINTS.count(point):
            filler.popleft()()
            k += 1

    def drain_boundary():
        k = 0
        while filler and k < 2:
            filler.popleft()()
            k += 1

    def flush_filler():
        while filler:
            filler.popleft()()

    # ---------------- attention slots ----------------
    wo_holder = []
    prefilled = set()
    ys_ev = [0]

    def oproj_group(key):
        tt, ob = key
        WOH, WOL = wo_holder[0]
        ps = p_q.tile([128, 512], F32, tag="q", bufs=1, name="ps_q")
        tb = tt * 128
        # hi-only (A) passes first so the group can start before the
        # wo_lo DMA lands.
        for half in range(2):
            off = half * 256
            o0 = ob * 512 + off
            for hp in range(HC // 2):
                nc.tensor.matmul(ps[:, off:off + 256],
                                 OTH[:, 2 * hp:2 * hp + 2, tb:tb + 128],
                                 WOH[:, 2 * hp:2 * hp + 2, o0:o0 + 256],
                                 start=(half == 0 and hp == 0), stop=False,
                                 perf_mode=DR)
        for half in range(2):
            off = half * 256
            o0 = ob * 512 + off
            for hp in range(HC // 2):
                nc.tensor.matmul(ps[:, off:off + 256],
                                 OTL[:, 2 * hp:2 * hp + 2, tb:tb + 128],
                                 WOH[:, 2 * hp:2 * hp + 2, o0:o0 + 256],
                                 start=False, stop=False, perf_mode=DR)
        for half in range(2):
            off = half * 256
            o0 = ob * 512 + off
            for hp in range(HC // 2):
                nc.tensor.matmul(ps[:, off:off + 256],
                                 OTH[:, 2 * hp:2 * hp + 2, tb:tb + 128],
                                 WOL[:, 2 * hp:2 * hp + 2, o0:o0 + 256],
                                 start=False,
                                 stop=(half == 1 and hp == HC // 2 - 1),
                                 perf_mode=DR)
        # y leaves scaled by 1/Y_SCALE and in bf16; the host multiplies
        # Y_SCALE into the partial-sum gather for free (bf16 also halves
        # the y DMA bytes).
        yt = st_pool.tile([128, 512], BF16, tag="yt8", bufs=2, name="yt")
        nc.any.tensor_copy(yt[:], ps[:])
        ys_ev[0] += 1
        nc.sync.dma_start(y_d[tt * 128:(tt + 1) * 128, ob * 512:ob * 512 + 512],
                          yt[:])
        prefilled.add(key)

    og_iter = iter([(tt, ob) for tt in range(3) for ob in range(4)][:11])
    with (
        tc.tile_pool(name="p_s", bufs=1, space="PSUM", side="right") as p_s,
        tc.tile_pool(name="p_u", bufs=1, space="PSUM", side="right") as p_u,
        tc.tile_pool(name="p_z", bufs=1, space="PSUM", side="right") as p_z,
    ):
        def make_slot(h, u):
            return {
                "h": h, "u": u, "g": h // 4, "i0": u * 1024,
                "U": p_u.tile([128, 1024], F32, tag="U", bufs=1, name="ps_U"),
                "Z": p_z.tile([128, 512], F32, tag="Z", bufs=1, name="ps_Z"),
                "Es": {}, "zred": [None, None],
            }

        def s_score(st, jt):
            ps = p_s.tile([128, 1024], F32, tag="S", bufs=2, name="ps_S")
            kt_sl = KT[:, st["g"], jt * 128:(jt + 1) * 128]
            i0 = st["i0"]
            nc.tensor.matmul(ps[:, 0:512], kt_sl,
                             QT[:, st["h"], i0:i0 + 512],
                             start=True, stop=True)
            nc.tensor.matmul(ps[:, 512:1024], kt_sl,
                             QT[:, st["h"], i0 + 512:i0 + 1024],
                             start=True, stop=True)
            E = e_pool.tile([128, 1024], BF16, tag="e", bufs=6, name="E")
            nc.scalar.activation(E[:], ps[:], Exp, scale=EXP_SCALE)
            st["Es"][jt] = E

        def s_av(st, jt):
            E = st["Es"][jt]
            v_sl = VV[:, jt, st["g"] * 128:(st["g"] + 1) * 128]
            s0, sp = (jt == 0), (jt == JT - 1)
            U_ps = st["U"]
            nc.tensor.matmul(U_ps[:, 0:512], v_sl, E[:, 0:512],
                             start=s0, stop=sp)
            nc.tensor.matmul(U_ps[:, 512:1024], v_sl, E[:, 512:1024],
                             start=s0, stop=sp)

        def s_z(st, jt):
            # Z column layout: [jt-half, ib, jt%8] so the first half's
            # reduce can run mid-slot and the next slot's writes never
            # collide with this slot's late reduce.
            E = st["Es"].pop(jt)
            Z_ps = st["Z"]
            col0 = (jt // 8) * 64 + (jt % 8)
            for ib in range(8):
                nc.tensor.matmul(
                    Z_ps[:, col0 + ib * 8:col0 + ib * 8 + 1],
                    E[:, ib * 128:(ib + 1) * 128], ones_t[:],
                    start=True, stop=True)

        def s_reduce_half(st, half):
            zr = rz_pool.tile([128, 8], F32, tag=f"zred{half}", bufs=1,
                              name="zred")
            nc.vector.tensor_reduce(
                zr[:],
                st["Z"][:, half * 64:half * 64 + 64].rearrange(
                    "p (a b) -> p a b", a=8),
                axis=AxX, op=Add)
            st["zred"][half] = zr

        def s_ztail(st):
            h, i0, U_ps, Z_ps = st["h"], st["i0"], st["U"], st["Z"]
            # Evict U to SBUF right away so the U psum bank is free
            # before the next slot's first AV matmul.
            # softmax denominator first on DVE (short ops) so the PE
            # transpose is unblocked before the 1.2us usb eviction runs.
            s_reduce_half(st, 1)
            zs = rz_pool.tile([128, 8], F32, tag="zs", bufs=1, name="zs")
            nc.vector.tensor_add(zs[:], st["zred"][0][:], st["zred"][1][:])
            rz = rz_pool.tile([128, 8], F32, tag="rz", bufs=1, name="rz")
            nc.vector.reciprocal(rz[:], zs[:])
            usb = rz_pool.tile([128, 1024], BF16, tag="usb", bufs=1,
                               name="usb")
            nc.any.tensor_copy(usb[:], U_ps[:])
            nc.tensor.matmul(Z_ps[0:8, 256:384], rz[:], ident_t[:],
                             start=True, stop=True, is_transpose=True)
            rzt = rz_pool.tile([8, 128], BF16, tag="rzt", bufs=1, name="rzt")
            nc.any.tensor_copy(rzt[:], Z_ps[0:8, 256:384])
            # flatten [8,128] -> [1,1024] via a DRAM round trip (plain
            # DMAs, off the critical path), then one gpsimd broadcast.
            # (hw partition_broadcast requires base partition 0, so the
            # 8-direct-broadcast variant fails BIR verification.)
            rsc = dram_pool.tile([8, 128], BF16, tag="rsc", bufs=2,
                                 name="rsc")
            nc.sync.dma_start(rsc[:], rzt[:])
            rzrow = rz_pool.tile([1, 1024], BF16, tag="rzrow", bufs=1,
                                 name="rzrow")
            nc.sync.dma_start(
                rzrow[:], rsc.rearrange("a b -> (a b)").unsqueeze(0))
            rzf = rz_pool.tile([128, 1024], BF16, tag="rzf", bufs=1,
                               name="rzf")
            nc.gpsimd.partition_broadcast(rzf[:], rzrow[0:1, :])
            tmp = rz_pool.tile([128, 1024], BF16, tag="ottmp", bufs=2,
                               name="ottmp")
            nc.gpsimd.tensor_mul(tmp[:], usb[:], rzf[:])
            nc.gpsimd.tensor_copy(OTH[:, h, i0:i0 + 1024], tmp[:])
            # Keep the lo-residual subtract on Pool: DVE executes in
            # order, and parking a late-resolving op there head-of-line
            # blocks the next slot's zred/usb chain (PE stalls on the
            # Z/U psum banks).
            nc.gpsimd.tensor_sub(OTL[:, h, i0:i0 + 1024], tmp[:],
                                 OTH[:, h, i0:i0 + 1024])

        # u-major slot order: all heads' first query half completes by
        # slot 7, so o_proj prefill groups (which only touch tt<8 token
        # blocks) stop depending on the LAST slot's late Pool chain.
        slots = [(h, u) for u in range(2) for h in range(HC)]
        cur = make_slot(0, 0)
        s_score(cur, 0)
        s_score(cur, 1)
        pushed = {0}
        for idx, (h, u) in enumerate(slots):
            if u == 0:
                prefetch_wq(h + 2)
                # supply one head ahead of the drain schedule so early
                # slots have filler slack (a rope-delayed chunk can't
                # starve the piece queue).
                for hh in ((1, 2) if h == 0 else (h + 2,)):
                    if hh < HC and hh not in pushed:
                        pushed.add(hh)
                        push_qproj(hh)
            st = cur
            nxt = None
            for jt in range(JT):
                s_av(st, jt)
                s_z(st, jt)
                if jt + 2 < JT:
                    s_score(st, jt + 2)
                elif idx + 1 < len(slots):
                    # Deep slot overlap: the next slot's first two scores
                    # go out at jt 14/15, reusing the psum bufs freed by
                    # exp(12)/exp(13), so exp_n(0) is already done when
                    # the next slot's first AV issues -- no boundary gap.
                    if nxt is None:
                        nxt = make_slot(*slots[idx + 1])
                    s_score(nxt, jt + 2 - JT)
                if jt == 8:
                    s_reduce_half(st, 0)
                if jt in PIECE_POINTS:
                    drain_at(jt)
                if ((idx == 14 and jt in (11, 13, 15))
                        or (idx == 15 and jt in (2, 5, 8, 11, 14))):
                    # the filler is exhausted by slot 13; fill the last
                    # two slots' ACT-paced slack with o_proj groups.
                    # slot 14's groups sit late enough that the wo DMAs
                    # (issued at slot 13's end) have landed.
                    oproj_group(next(og_iter))
            if idx + 1 < len(slots):
                cur = nxt
                if filler:
                    drain_boundary()
                elif idx >= 14:
                    oproj_group(next(og_iter))
            else:
                # keep the PE busy while the final denominator chain
                # (zred/zs/rz on DVE) runs, so the ztail transpose that
                # precedes the o_proj phase in the in-order PE stream
                # doesn't stall it.
                oproj_group(next(og_iter))
                oproj_group(next(og_iter))
            s_ztail(st)
            if idx == 13:
                # X no longer needed (last q-projection drained); swap the
                # X heap space for the wo hi/lo tiles. wo_hi first: the
                # prefill groups' A-passes only need it + OT.
                flush_filler()
                x_ctx.close()
                wo_pool = ctx.enter_context(
                    tc.tile_pool(name="wo", bufs=1, side="left"))
                WOH = wo_pool.tile([128, HC, S], FP8, tag="woh", bufs=1,
                                   name="woh_t")
                WOL = wo_pool.tile([128, HC, S], FP8, tag="wol", bufs=1,
                                   name="wol_t")
                wo_holder.append((WOH, WOL))
                # one big strided DMA per tensor: same bytes, but 2 HWDGE
                # generations instead of 16 (~4us less queue time), so the
                # slot-14 o_proj prefill groups unblock earlier.
                nc.sync.dma_start(WOH[:], woh_d.rearrange("h p s -> p h s"))
                wol_r = wol_d.rearrange("h p s -> p h s")
                nc.sync.dma_start(WOL[:, 0:HC // 2, :], wol_r[:, 0:HC // 2, :])
                nc.sync.dma_start(WOL[:, HC // 2:, :], wol_r[:, HC // 2:, :])

    pq_ctx.close()

    # ---------------- output projection ----------------
    WOH, WOL = wo_holder[0]
    with (
        tc.tile_pool(name="p_y", bufs=1, space="PSUM", side="right") as p_y,
        tc.tile_pool(name="ys", bufs=1, side="right") as ys_pool,
    ):
        ev = 0
        remaining = [(tt, ob) for tt in range(16) for ob in range(4)
                     if (tt, ob) not in prefilled]
        for gi, (tt, ob) in enumerate(remaining):
            tb = tt * 128
            if gi == len(remaining) - 1:
                # final group: two independent psum half-tiles, each
                # evicted + DMA'd as soon as its matmuls finish, so the
                # post-PE drain is one small eviction + DMA.
                for half in range(2):
                    o0 = ob * 512 + half * 256
                    psh = p_y.tile([128, 256], F32, tag="yl", bufs=2,
                                   name="ps_yl")
                    for hp in range(HC // 2):
                        dr3(psh[:],
                            OTH[:, 2 * hp:2 * hp + 2, tb:tb + 128],
                            OTL[:, 2 * hp:2 * hp + 2, tb:tb + 128],
                            WOH[:, 2 * hp:2 * hp + 2, o0:o0 + 256],
                            WOL[:, 2 * hp:2 * hp + 2, o0:o0 + 256],
                            first=(hp == 0), last=(hp == HC // 2 - 1))
                    yth = ys_pool.tile([128, 256], BF16, tag="ytl", bufs=2,
                                       name="ytl")
                    if half == 0:
                        nc.scalar.copy(yth[:], psh[:])
                        nc.scalar.dma_start(y_d[tb:tb + 128, o0:o0 + 256],
                                            yth[:])
                    else:
                        nc.vector.tensor_copy(yth[:], psh[:])
                        nc.sync.dma_start(y_d[tb:tb + 128, o0:o0 + 256],
                                          yth[:])
                continue
            ps = p_y.tile([128, 512], F32, tag="y", bufs=6, name="ps_y")
            for half in range(2):
                off = half * 256
                o0 = ob * 512 + off
                for hp in range(HC // 2):
                    dr3(ps[:, off:off + 256],
                        OTH[:, 2 * hp:2 * hp + 2, tb:tb + 128],
                        OTL[:, 2 * hp:2 * hp + 2, tb:tb + 128],
                        WOH[:, 2 * hp:2 * hp + 2, o0:o0 + 256],
                        WOL[:, 2 * hp:2 * hp + 2, o0:o0 + 256],
                        first=(half == 0 and hp == 0),
                        last=(half == 1 and hp == HC // 2 - 1))
            yt = ys_pool.tile([128, 512], BF16, tag="yt", bufs=8,
                              name="yt")
            nc.any.tensor_copy(yt[:], ps[:])
            ev += 1
            nc.sync.dma_start(
                y_d[tt * 128:(tt + 1) * 128, ob * 512:ob * 512 + 512],
                yt[:])

    ctx.close()


def _build(repeat=1):
    import concourse.mybir as mybir
    import concourse.tile as tile
    from concourse import bacc

    F32 = mybir.dt.float32
    BF16 = mybir.dt.bfloat16
    FP8 = mybir.dt.float8e4

    nc = bacc.Bacc("TRN2", target_bir_lowering=False, debug=False)
    # X chunked [tch, 128, CT, 256] so each 256-token chunk DMA has 4KB
    # contiguous per-partition runs (sub-512B descriptors pay 2x on the
    # DMA bus).
    xh_d = nc.dram_tensor("xh", [8, 128, CT, 256], FP8,
                          kind="ExternalInput").ap()
    xl_d = nc.dram_tensor("xl", [8, 128, CT, 256], FP8,
                          kind="ExternalInput").ap()
    cosk_d = nc.dram_tensor("cosk", [128, S], BF16, kind="ExternalInput").ap()
    sinkm_d = nc.dram_tensor("sinkm", [128, S], BF16,
                             kind="ExternalInput").ap()
    wqh_d = nc.dram_tensor("wqh", [HC, 128, CT, 128], FP8,
                           kind="ExternalInput").ap()
    wql_d = nc.dram_tensor("wql", [HC, 128, CT, 128], FP8,
                           kind="ExternalInput").ap()
    wkh_d = nc.dram_tensor("wkh", [KVC, 128, CT, 128], FP8,
                           kind="ExternalInput").ap()
    wkl_d = nc.dram_tensor("wkl", [KVC, 128, CT, 128], FP8,
                           kind="ExternalInput").ap()
    wvh_d = nc.dram_tensor("wvh", [128, CT, 256], FP8,
                           kind="ExternalInput").ap()
    wvl_d = nc.dram_tensor("wvl", [128, CT, 256], FP8,
                           kind="ExternalInput").ap()
    woh_d = nc.dram_tensor("woh", [HC, 128, S], FP8, kind="ExternalInput").ap()
    wol_d = nc.dram_tensor("wol", [HC, 128, S], FP8, kind="ExternalInput").ap()
    ones_d = nc.dram_tensor("ones", [128, 1], BF16, kind="ExternalInput").ap()
    ident_d = nc.dram_tensor("ident", [128, 128], F32,
                             kind="ExternalInput").ap()
    y_d = nc.dram_tensor("y", [S, HID], BF16, kind="ExternalOutput").ap()

    with tile.TileContext(nc) as tc:
        for _ in range(repeat):
            _emit(nc, tc, (xh_d, xl_d, cosk_d, sinkm_d, wqh_d, wql_d, wkh_d,
                           wkl_d, wvh_d, wvl_d, woh_d, wol_d, ones_d, ident_d,
                           y_d))
    nc.compile()
    return nc


class _Runner:
    """Persistent-jit PJRT executor (axon) / NRT executor (native)."""

    def __init__(self, nc):
        self.nc = nc
        from concourse._compat import axon_active
        self.axon = axon_active()
        if not self.axon:
            return
        import jax
        from jax.sharding import Mesh, PartitionSpec
        from jax.experimental.shard_map import shard_map
        import concourse.mybir as mybir
        from concourse.bass2jax import (
            _bass_exec_p, install_neuronx_cc_hook, partition_id_tensor)

        install_neuronx_cc_hook()
        partition_name = (nc.partition_id_tensor.name
                          if nc.partition_id_tensor else None)
        in_names, out_names, out_avals, zero_outs = [], [], [], []
        for alloc in nc.m.functions[0].allocations:
            if not isinstance(alloc, mybir.MemoryLocationSet):
                continue
            name = alloc.memorylocations[0].name
            if alloc.kind == "ExternalInput":
                if name != partition_name:
                    in_names.append(name)
            elif alloc.kind == "ExternalOutput":
                shape = tuple(alloc.tensor_shape)
                dtype = mybir.dt.np(alloc.dtype)
                out_names.append(name)
                out_avals.append(jax.core.ShapedArray(shape, dtype))
                zero_outs.append(np.zeros(shape, dtype))
        self.in_names, self.out_names = in_names, out_names
        self.zero_outs = zero_outs
        n_params, n_outs = len(in_names), len(out_names)
        all_in = in_names + out_names
        if partition_name is not None:
            all_in.append(partition_name)
        donate = tuple(range(n_params, n_params + n_outs))

        def _body(*args):
            operands = list(args)
            if partition_name is not None:
                operands.append(partition_id_tensor())
            return tuple(_bass_exec_p.bind(
                *operands,
                out_avals=tuple(out_avals),
                in_names=tuple(all_in),
                out_names=tuple(out_names),
                lowering_input_output_aliases=(),
                sim_require_finite=True,
                sim_require_nnan=True,
                nc=nc,
            ))

        devices = jax.devices()[:N_CORES]
        mesh = Mesh(np.asarray(devices), ("core",))
        self._fn = jax.jit(
            shard_map(_body, mesh=mesh,
                      in_specs=(PartitionSpec("core"),) * (n_params + n_outs),
                      out_specs=(PartitionSpec("core"),) * n_outs,
                      check_rep=False),
            donate_argnums=donate, keep_unused=True,
        )

    def run(self, in_maps):
        if not self.axon:
            from concourse import bass_utils
            res = bass_utils.run_bass_kernel_spmd(
                self.nc, in_maps, core_ids=list(range(N_CORES)))
            return res.results
        concat_in = [
            np.concatenate([np.asarray(in_maps[c][n]) for c in range(N_CORES)],
                           axis=0)
            for n in self.in_names
        ] + [np.concatenate([z] * N_CORES, axis=0) for z in self.zero_outs]
        outs = [np.asarray(o) for o in self._fn(*concat_in)]
        per_core = []
        for c in range(N_CORES):
            d = {}
            for name, o in zip(self.out_names, outs):
                rows = o.shape[0] // N_CORES
                d[name] = o[c * rows:(c + 1) * rows]
            per_core.append(d)
        return per_core


def _prep_inputs(x, cos, sin, wq, wk, wv, wo):
    import concourse.mybir as mybir
    f32 = np.float32
    bf16 = mybir.dt.np(mybir.dt.bfloat16)
    fp8 = mybir.dt.np(mybir.dt.float8e4)

    def split8(arr, scale):
        s = np.ascontiguousarray(arr, f32) * f32(scale)
        hi = s.astype(fp8)
        lo = (s - hi.astype(f32)).astype(fp8)
        return hi, lo

    cosT = np.asarray(cos).T.astype(f32)    # [128, S]
    # half-swapped sin table: row r holds the sin factor that multiplies
    # q[row r] when producing the OTHER half's rotate_half term, so both
    # tensor_mul inputs share a base partition (hw BIR constraint).
    sinT = np.asarray(sin).T.astype(f32)
    sinm = np.concatenate([sinT[64:128], -sinT[0:64]], axis=0).copy()
    cosT = np.ascontiguousarray(cosT).astype(bf16)
    sinm = np.ascontiguousarray(sinm).astype(bf16)
    ones = np.full((128, 1), 1.0 / EO, bf16)
    ident = np.eye(128, dtype=f32)
    x = np.asarray(x, f32)
    wq = np.asarray(wq, f32)
    wk = np.asarray(wk, f32)
    wv = np.asarray(wv, f32)
    wo = np.asarray(wo, f32)

    in_maps = []
    for c in range(N_CORES):
        b, kh = c // 2, c % 2
        # X^T packed [tch, p, ct, tok%256]: [p, ct, j] = x[b, j, ct*128+p]
        xt = x[b].T.reshape(CT, 128, S).transpose(1, 0, 2)
        xt = xt.reshape(128, CT, 8, 256).transpose(2, 0, 1, 3)
        xh, xl = split8(xt, AX)
        # wq rows for this core's heads -> [h, p(ct-part), ct, c(col)]
        wq_c = wq[kh * 1024:(kh + 1) * 1024, :]
        wqt = wq_c.reshape(HC, 128, CT, 128).transpose(0, 3, 2, 1)
        wqh, wql = split8(wqt, BW)
        wk_c = wk[kh * 256:(kh + 1) * 256, :]
        wkt = wk_c.reshape(KVC, 128, CT, 128).transpose(0, 3, 2, 1)
        wkh, wkl = split8(wkt, BW)
        wv_c = wv[kh * 256:(kh + 1) * 256, :]
        wvt = wv_c.reshape(256, CT, 128).transpose(2, 1, 0)
        wvh, wvl = split8(wvt, BW)
        # wo columns for this core's heads -> [h, p(=d), out]
        wot = wo[:, kh * 1024:(kh + 1) * 1024].T.reshape(HC, 128, S)
        woh, wol = split8(wot, GW)
        in_maps.append({
            "xh": xh, "xl": xl, "cosk": cosT, "sinkm": sinm,
            "wqh": wqh, "wql": wql, "wkh": wkh, "wkl": wkl,
            "wvh": wvh, "wvl": wvl, "woh": woh, "wol": wol,
            "ones": ones, "ident": ident,
        })
    return in_maps


def kernel(x, cos, sin, wq, wk, wv, wo):
    if "nc" not in _cache:
        _cache["nc"] = _build()
        _cache["runner"] = _Runner(_cache["nc"])
    runner = _cache["runner"]
    in_maps = _prep_inputs(x, cos, sin, wq, wk, wv, wo)
    results = runner.run(in_maps)
    y = np.empty((B, S, HID), np.float32)
    for b in range(B):
        y[b] = (results[2 * b]["y"].astype(np.float32)
                + results[2 * b + 1]["y"].astype(np.float32)) * np.float32(
                    Y_SCALE)
    return y
